# revision 6
# baseline (speedup 1.0000x reference)
"""Trainium2 Bass kernel for a 3x3 'same' conv: x [8,16,512,512] f32, weight [16,144].

Data-parallel over batch: 1 image per NeuronCore, 8 cores.

Design (v2 -- large-DMA pipeline):
  - Host pre-gathers x into group-window layout xp[ci*8+j, g, c] (f16,
    514 padded cols, zero rows baked in), so each group's moving operand
    is a plain SBUF slice and every DMA line is >=2KB contiguous per
    partition.  Groups g cover output rows y0(g)=6g (g<85) / 506 (g=85);
    window = padded rows 6g..6g+7.
  - Input arrives via 8 graduated dma_starts on the sync HWDGE queue
    (265KB..2.1MB); whole image stays resident in SBUF (88KB/partition).
  - One stationary set [128, 3*96]: wk[ci*8+(r+kh), kw*96+co*6+r] =
    w[co,ci,kh,kw].  Per group: 3 accumulating matmuls (kw taps, moving
    slice [kw, kw+512)) into one PSUM bank [96, 512] f32.  Batches of 4
    groups issued kw-major so consecutive matmuls share a stationary.
  - PSUM evacuation split across engines: even groups on ScalarE
    ((172+512)/1.2 = 570ns), odd on VectorE ((120+512)/0.96 = 658ns),
    casting f32->f16 into a staging tile [96, 8, 512].
  - Output leaves via 12 chunked dma_starts on the scalar HWDGE queue
    ([96, <=8, 512] f16, <=790KB); host scatters rows back and upcasts.

Roofline: PE 258 matmuls x 512 cols at 2.4GHz = 55us; HBM 20MB at
~358GB/s = 56us; DVE/ACT ~30us each.  Target span ~60us.
"""

from contextlib import ExitStack

import numpy as np

C_OUT, C_IN, KH, KW = 16, 16, 3, 3
H = W = 512
WP = W + 2      # padded row length
B = 8
R = 6           # output rows per group
J = R + 2       # input rows per group
M = C_OUT * R   # 96 psum partitions
K = C_IN * J    # 128 contraction partitions
NG = 86
Y0 = [6 * g for g in range(85)] + [506]

IN_CHUNKS = [2, 4, 6, 10, 16, 16, 16, 16]    # groups per input dma_start
OUT_CHUNKS = [8] * 10 + [3, 2, 1]            # groups per output dma_start
GB = 4                                       # groups per matmul batch
NDUMMY = 6                                   # PE-warmup matmuls during DMA wait

_CACHE = {}


def _build_weights(weight: np.ndarray) -> np.ndarray:
    """[16,144] -> [128, 3*96]: wk[ci*8+(r+kh), kw*96+co*6+r] = w[co,ci,kh,kw]."""
    w = np.asarray(weight, dtype=np.float32).reshape(C_OUT, C_IN, KH, KW)
    wk = np.zeros((K, KW, M), np.float32)
    for kw in range(KW):
        for kh in range(KH):
            for r in range(R):
                j = r + kh
                for co in range(C_OUT):
                    for ci in range(C_IN):
                        wk[ci * J + j, kw, co * R + r] = w[co, ci, kh, kw]
    return np.ascontiguousarray(wk.reshape(K, KW * M)).astype(np.float16)


def _build_x(x: np.ndarray) -> np.ndarray:
    """[16,512,512] f32 -> [128, 86, 514] f16 group-window gather."""
    xpad = np.zeros((C_IN, H + 2, WP), np.float16)
    xpad[:, 1 : H + 1, 1 : W + 1] = x.astype(np.float16)
    starts = np.array([y0 for y0 in Y0])          # padded-row start of window
    idx = starts[:, None] + np.arange(J)[None, :]  # (86, 8) in [0, 513]
    xp = xpad[:, idx, :]                           # (16, 86, 8, 514)
    xp = np.ascontiguousarray(xp.transpose(0, 2, 1, 3)).reshape(C_IN * J, NG, WP)
    return xp


def _build_nc():
    import concourse.tile as tile
    from concourse import bacc, mybir

    f32 = mybir.dt.float32
    f16 = mybir.dt.float16

    nc = bacc.Bacc("TRN2", target_bir_lowering=False, debug=False,
                   enable_asserts=False, num_devices=B)
    x = nc.dram_tensor("x", [K, NG, WP], f16, kind="ExternalInput").ap()
    wkin = nc.dram_tensor("wk", [K, KW * M], f16, kind="ExternalInput").ap()
    out = nc.dram_tensor("out", [M, NG, W], f16, kind="ExternalOutput").ap()

    in_start = np.cumsum([0] + IN_CHUNKS)
    out_start = np.cumsum([0] + OUT_CHUNKS)
    chunk_of = np.searchsorted(in_start, np.arange(NG), side="right") - 1
    ochunk_of = np.searchsorted(out_start, np.arange(NG), side="right") - 1

    with tile.TileContext(nc) as tc, ExitStack() as ctx:
        wpool = ctx.enter_context(tc.tile_pool(name="wpool", bufs=1))
        xpools = [ctx.enter_context(tc.tile_pool(name=f"xp{i}", bufs=1))
                  for i in range(len(IN_CHUNKS))]
        opool = ctx.enter_context(tc.tile_pool(name="opool", bufs=3))
        ppool = ctx.enter_context(tc.tile_pool(name="ppool", bufs=8, space="PSUM"))

        wt = wpool.tile([K, KW * M], f16, name="wt")
        dummy = wpool.tile([K, W], f16, name="dummy")
        nc.gpsimd.memset(dummy[:], 0)
        nc.sync.dma_start(out=wt[:], in_=wkin[:])

        xts = []
        for i, ng in enumerate(IN_CHUNKS):
            g0 = int(in_start[i])
            xt = xpools[i].tile([K, ng, WP], f16, name=f"xt{i}")
            nc.sync.dma_start(out=xt[:], in_=x[:, g0 : g0 + ng, :])
            xts.append(xt)

        # warm the PE (HAM un-throttles after ~3.4us of sustained activity)
        # while the first input chunk + weights are still in flight
        pd = ppool.tile([M, W], f32, name="pt", tag="pt")
        for _ in range(NDUMMY):
            nc.tensor.matmul(pd[:, 0:W], dummy[:, 0:M], dummy[:, 0:W],
                             start=True, stop=True)

        ot = None
        for b in range(0, NG, GB):
            groups = list(range(b, min(b + GB, NG)))
            pts = [ppool.tile([M, W], f32, name="pt", tag="pt") for _ in groups]
            for kw in range(KW):
                for g, pt in zip(groups, pts):
                    c = int(chunk_of[g])
                    gi = g - int(in_start[c])
                    nc.tensor.matmul(pt[:, 0:W], wt[:, kw * M : (kw + 1) * M],
                                     xts[c][:, gi, kw : kw + W],
                                     start=(kw == 0), stop=(kw == KW - 1))
            for g, pt in zip(groups, pts):
                oc = int(ochunk_of[g])
                o0 = int(out_start[oc])
                if ot is None:
                    ot = opool.tile([M, OUT_CHUNKS[0], W], f16, name="ot", tag="ot")
                oi = g - o0
                if g % 2 == 0:
                    nc.scalar.copy(ot[:, oi, :], pt[:])
                else:
                    nc.vector.tensor_copy(ot[:, oi, :], pt[:])
                if g == o0 + OUT_CHUNKS[oc] - 1:
                    nc.scalar.dma_start(out=out[:, o0 : o0 + OUT_CHUNKS[oc], :],
                                        in_=ot[:, 0 : OUT_CHUNKS[oc], :])
                    ot = None

    nc.compile()
    return nc


def get_nc():
    if "v2" not in _CACHE:
        _CACHE["v2"] = _build_nc()
    return _CACHE["v2"]


def run(x: np.ndarray, weight: np.ndarray, **spmd_kwargs):
    """Run the conv on 8 cores; returns (out [8,16,512,512] f32, results)."""
    from concourse.bass_utils import run_bass_kernel_spmd

    x = np.asarray(x, dtype=np.float32)
    wk = _build_weights(weight)
    xps = [_build_x(x[b]) for b in range(B)]
    nc = get_nc()
    in_maps = [{"x": xps[b], "wk": wk} for b in range(B)]
    res = run_bass_kernel_spmd(nc, in_maps, list(range(B)), **spmd_kwargs)

    rows = np.array(Y0)[:, None] + np.arange(R)[None, :]   # (86, 6)
    out = np.empty((B, C_OUT, H, W), np.float32)
    for b in range(B):
        o = res.results[b]["out"].astype(np.float32)       # [96, 86, 512]
        o = o.reshape(C_OUT, R, NG, W).transpose(0, 2, 1, 3)  # [16, 86, 6, 512]
        out[b][:, rows.reshape(-1), :] = o.reshape(C_OUT, NG * R, W)
    return out, res


def kernel(x: np.ndarray, weight: np.ndarray) -> np.ndarray:
    return run(x, weight)[0]


# revision 8
# speedup vs baseline: 1.0965x; 1.0965x over previous
"""Trainium2 Bass kernel for a 3x3 'same' conv: x [8,16,512,512] f32, weight [16,144].

Data-parallel over batch: 1 image per NeuronCore, 8 cores.

Design (v2 -- large-DMA pipeline):
  - Host pre-gathers x into group-window layout xp[ci*8+j, g, c] (f16,
    514 padded cols, zero rows baked in), so each group's moving operand
    is a plain SBUF slice and every DMA line is >=2KB contiguous per
    partition.  Groups g cover output rows y0(g)=6g (g<85) / 506 (g=85);
    window = padded rows 6g..6g+7.
  - Input arrives via 8 graduated dma_starts on the sync HWDGE queue
    (265KB..2.1MB); whole image stays resident in SBUF (88KB/partition).
  - One stationary set [128, 3*96]: wk[ci*8+(r+kh), kw*96+co*6+r] =
    w[co,ci,kh,kw].  Per group: 3 accumulating matmuls (kw taps, moving
    slice [kw, kw+512)) into one PSUM bank [96, 512] f32.  Batches of 4
    groups issued kw-major so consecutive matmuls share a stationary.
  - PSUM evacuation split across engines: even groups on ScalarE
    ((172+512)/1.2 = 570ns), odd on VectorE ((120+512)/0.96 = 658ns),
    casting f32->f16 into a staging tile [96, 8, 512].
  - Output leaves via 12 chunked dma_starts on the scalar HWDGE queue
    ([96, <=8, 512] f16, <=790KB); host scatters rows back and upcasts.

Roofline: PE 258 matmuls x 512 cols at 2.4GHz = 55us; HBM 20MB at
~358GB/s = 56us; DVE/ACT ~30us each.  Target span ~60us.
"""

from contextlib import ExitStack

import numpy as np

C_OUT, C_IN, KH, KW = 16, 16, 3, 3
H = W = 512
WP = W + 2      # padded row length
B = 8
R = 6           # output rows per group
J = R + 2       # input rows per group
M = C_OUT * R   # 96 psum partitions
K = C_IN * J    # 128 contraction partitions
NG = 86
Y0 = [6 * g for g in range(85)] + [506]

# gentle ladder: each chunk <=1.25x previous so a PE catch-up stall is
# always shorter than the ~3.4us HAM re-throttle window
IN_CHUNKS = [2, 2, 3, 3, 4, 5, 6, 7, 8, 10, 12, 12, 12]
OUT_CHUNKS = [8] * 10 + [3, 2, 1]            # groups per output dma_start
OUT_SYNC = 2                                 # final N output chunks go via sync queue
GB = 4                                       # groups per matmul batch
NDUMMY = 11                                  # PE-warmup matmuls during DMA wait

_CACHE = {}


def _build_weights(weight: np.ndarray) -> np.ndarray:
    """[16,144] -> [128, 3*96]: wk[ci*8+(r+kh), kw*96+co*6+r] = w[co,ci,kh,kw]."""
    w = np.asarray(weight, dtype=np.float32).reshape(C_OUT, C_IN, KH, KW)
    wk = np.zeros((K, KW, M), np.float32)
    for kw in range(KW):
        for kh in range(KH):
            for r in range(R):
                j = r + kh
                for co in range(C_OUT):
                    for ci in range(C_IN):
                        wk[ci * J + j, kw, co * R + r] = w[co, ci, kh, kw]
    return np.ascontiguousarray(wk.reshape(K, KW * M)).astype(np.float16)


def _build_x(x: np.ndarray) -> np.ndarray:
    """[16,512,512] f32 -> [128, 86, 514] f16 group-window gather."""
    xpad = np.zeros((C_IN, H + 2, WP), np.float16)
    xpad[:, 1 : H + 1, 1 : W + 1] = x.astype(np.float16)
    starts = np.array([y0 for y0 in Y0])          # padded-row start of window
    idx = starts[:, None] + np.arange(J)[None, :]  # (86, 8) in [0, 513]
    xp = xpad[:, idx, :]                           # (16, 86, 8, 514)
    xp = np.ascontiguousarray(xp.transpose(0, 2, 1, 3)).reshape(C_IN * J, NG, WP)
    return xp


def _build_nc():
    import concourse.tile as tile
    from concourse import bacc, mybir

    f32 = mybir.dt.float32
    f16 = mybir.dt.float16

    nc = bacc.Bacc("TRN2", target_bir_lowering=False, debug=False,
                   enable_asserts=False, num_devices=B)
    x = nc.dram_tensor("x", [K, NG, WP], f16, kind="ExternalInput").ap()
    wkin = nc.dram_tensor("wk", [K, KW * M], f16, kind="ExternalInput").ap()
    out = nc.dram_tensor("out", [M, NG, W], f16, kind="ExternalOutput").ap()

    in_start = np.cumsum([0] + IN_CHUNKS)
    out_start = np.cumsum([0] + OUT_CHUNKS)
    chunk_of = np.searchsorted(in_start, np.arange(NG), side="right") - 1
    ochunk_of = np.searchsorted(out_start, np.arange(NG), side="right") - 1

    with tile.TileContext(nc) as tc, ExitStack() as ctx:
        wpool = ctx.enter_context(tc.tile_pool(name="wpool", bufs=1))
        xpools = [ctx.enter_context(tc.tile_pool(name=f"xp{i}", bufs=1))
                  for i in range(len(IN_CHUNKS))]
        opool = ctx.enter_context(tc.tile_pool(name="opool", bufs=3))
        ppool = ctx.enter_context(tc.tile_pool(name="ppool", bufs=8, space="PSUM"))

        wt = wpool.tile([K, KW * M], f16, name="wt")
        dummy = wpool.tile([K, W], f16, name="dummy")
        nc.gpsimd.memset(dummy[:], 0)
        nc.sync.dma_start(out=wt[:], in_=wkin[:])

        xts = []
        for i, ng in enumerate(IN_CHUNKS):
            g0 = int(in_start[i])
            xt = xpools[i].tile([K, ng, WP], f16, name=f"xt{i}")
            nc.sync.dma_start(out=xt[:], in_=x[:, g0 : g0 + ng, :])
            xts.append(xt)

        # warm the PE (HAM un-throttles after ~3.4us of sustained activity)
        # while the first input chunk + weights are still in flight
        pd = ppool.tile([M, W], f32, name="pt", tag="pt")
        for _ in range(NDUMMY):
            nc.tensor.matmul(pd[:, 0:W], dummy[:, 0:M], dummy[:, 0:W],
                             start=True, stop=True)

        ot = None
        for b in range(0, NG, GB):
            groups = list(range(b, min(b + GB, NG)))
            pts = [ppool.tile([M, W], f32, name="pt", tag="pt") for _ in groups]
            for kw in range(KW):
                for g, pt in zip(groups, pts):
                    c = int(chunk_of[g])
                    gi = g - int(in_start[c])
                    nc.tensor.matmul(pt[:, 0:W], wt[:, kw * M : (kw + 1) * M],
                                     xts[c][:, gi, kw : kw + W],
                                     start=(kw == 0), stop=(kw == KW - 1))
            for g, pt in zip(groups, pts):
                oc = int(ochunk_of[g])
                o0 = int(out_start[oc])
                if ot is None:
                    ot = opool.tile([M, OUT_CHUNKS[0], W], f16, name="ot", tag="ot")
                oi = g - o0
                if g % 2 == 0:
                    nc.scalar.copy(ot[:, oi, :], pt[:])
                else:
                    nc.vector.tensor_copy(ot[:, oi, :], pt[:])
                if g == o0 + OUT_CHUNKS[oc] - 1:
                    oeng = nc.sync if oc >= len(OUT_CHUNKS) - OUT_SYNC else nc.scalar
                    oeng.dma_start(out=out[:, o0 : o0 + OUT_CHUNKS[oc], :],
                                   in_=ot[:, 0 : OUT_CHUNKS[oc], :])
                    ot = None

    nc.compile()
    return nc


def get_nc():
    if "v2" not in _CACHE:
        _CACHE["v2"] = _build_nc()
    return _CACHE["v2"]


def run(x: np.ndarray, weight: np.ndarray, **spmd_kwargs):
    """Run the conv on 8 cores; returns (out [8,16,512,512] f32, results)."""
    from concourse.bass_utils import run_bass_kernel_spmd

    x = np.asarray(x, dtype=np.float32)
    wk = _build_weights(weight)
    xps = [_build_x(x[b]) for b in range(B)]
    nc = get_nc()
    in_maps = [{"x": xps[b], "wk": wk} for b in range(B)]
    res = run_bass_kernel_spmd(nc, in_maps, list(range(B)), **spmd_kwargs)

    rows = np.array(Y0)[:, None] + np.arange(R)[None, :]   # (86, 6)
    out = np.empty((B, C_OUT, H, W), np.float32)
    for b in range(B):
        o = res.results[b]["out"].astype(np.float32)       # [96, 86, 512]
        o = o.reshape(C_OUT, R, NG, W).transpose(0, 2, 1, 3)  # [16, 86, 6, 512]
        out[b][:, rows.reshape(-1), :] = o.reshape(C_OUT, NG * R, W)
    return out, res


def kernel(x: np.ndarray, weight: np.ndarray) -> np.ndarray:
    return run(x, weight)[0]


# revision 10
# speedup vs baseline: 1.1173x; 1.0190x over previous
"""Trainium2 Bass kernel for a 3x3 'same' conv: x [8,16,512,512] f32, weight [16,144].

Data-parallel over batch: 1 image per NeuronCore, 8 cores.

Design (v2 -- large-DMA pipeline):
  - Host pre-gathers x into group-window layout xp[ci*8+j, g, c] (f16,
    514 padded cols, zero rows baked in), so each group's moving operand
    is a plain SBUF slice and every DMA line is >=2KB contiguous per
    partition.  Groups g cover output rows y0(g)=6g (g<85) / 506 (g=85);
    window = padded rows 6g..6g+7.
  - Input arrives via 8 graduated dma_starts on the sync HWDGE queue
    (265KB..2.1MB); whole image stays resident in SBUF (88KB/partition).
  - One stationary set [128, 3*96]: wk[ci*8+(r+kh), kw*96+co*6+r] =
    w[co,ci,kh,kw].  Per group: 3 accumulating matmuls (kw taps, moving
    slice [kw, kw+512)) into one PSUM bank [96, 512] f32.  Batches of 4
    groups issued kw-major so consecutive matmuls share a stationary.
  - PSUM evacuation split across engines: even groups on ScalarE
    ((172+512)/1.2 = 570ns), odd on VectorE ((120+512)/0.96 = 658ns),
    casting f32->f16 into a staging tile [96, 8, 512].
  - Output leaves via 12 chunked dma_starts on the scalar HWDGE queue
    ([96, <=8, 512] f16, <=790KB); host scatters rows back and upcasts.

Roofline: PE 258 matmuls x 512 cols at 2.4GHz = 55us; HBM 20MB at
~358GB/s = 56us; DVE/ACT ~30us each.  Target span ~60us.
"""

from contextlib import ExitStack

import numpy as np

C_OUT, C_IN, KH, KW = 16, 16, 3, 3
H = W = 512
WP = W + 2      # padded row length
B = 8
R = 6           # output rows per group
J = R + 2       # input rows per group
M = C_OUT * R   # 96 psum partitions
K = C_IN * J    # 128 contraction partitions
NG = 86
Y0 = [6 * g for g in range(85)] + [506]

# gentle ladder: each chunk <=1.25x previous so a PE catch-up stall is
# always shorter than the ~3.4us HAM re-throttle window
IN_CHUNKS = [2, 2, 3, 3, 4, 5, 6, 7, 8, 10, 12, 12, 12]
OUT_CHUNKS = [8] * 10 + [3, 2, 1]            # groups per output dma_start
OUT_SYNC = 0                                 # final N output chunks go via sync queue
GB = 4                                       # groups per matmul batch
NDUMMY = 9                                   # PE-warmup matmuls during DMA wait

_CACHE = {}


def _build_weights(weight: np.ndarray) -> np.ndarray:
    """[16,144] -> [128, 3*96]: wk[ci*8+(r+kh), kw*96+co*6+r] = w[co,ci,kh,kw]."""
    w = np.asarray(weight, dtype=np.float32).reshape(C_OUT, C_IN, KH, KW)
    wk = np.zeros((K, KW, M), np.float32)
    for kw in range(KW):
        for kh in range(KH):
            for r in range(R):
                j = r + kh
                for co in range(C_OUT):
                    for ci in range(C_IN):
                        wk[ci * J + j, kw, co * R + r] = w[co, ci, kh, kw]
    return np.ascontiguousarray(wk.reshape(K, KW * M)).astype(np.float16)


def _build_x(x: np.ndarray) -> np.ndarray:
    """[16,512,512] f32 -> [128, 86, 514] f16 group-window gather."""
    xpad = np.zeros((C_IN, H + 2, WP), np.float16)
    xpad[:, 1 : H + 1, 1 : W + 1] = x.astype(np.float16)
    starts = np.array([y0 for y0 in Y0])          # padded-row start of window
    idx = starts[:, None] + np.arange(J)[None, :]  # (86, 8) in [0, 513]
    xp = xpad[:, idx, :]                           # (16, 86, 8, 514)
    xp = np.ascontiguousarray(xp.transpose(0, 2, 1, 3)).reshape(C_IN * J, NG, WP)
    return xp


def _build_nc():
    import concourse.tile as tile
    from concourse import bacc, mybir

    f32 = mybir.dt.float32
    f16 = mybir.dt.float16

    nc = bacc.Bacc("TRN2", target_bir_lowering=False, debug=False,
                   enable_asserts=False, num_devices=B)
    x = nc.dram_tensor("x", [K, NG, WP], f16, kind="ExternalInput").ap()
    wkin = nc.dram_tensor("wk", [K, KW * M], f16, kind="ExternalInput").ap()
    out = nc.dram_tensor("out", [M, NG, W], f16, kind="ExternalOutput").ap()

    in_start = np.cumsum([0] + IN_CHUNKS)
    out_start = np.cumsum([0] + OUT_CHUNKS)
    chunk_of = np.searchsorted(in_start, np.arange(NG), side="right") - 1
    ochunk_of = np.searchsorted(out_start, np.arange(NG), side="right") - 1

    with tile.TileContext(nc) as tc, ExitStack() as ctx:
        wpool = ctx.enter_context(tc.tile_pool(name="wpool", bufs=1))
        xpools = [ctx.enter_context(tc.tile_pool(name=f"xp{i}", bufs=1))
                  for i in range(len(IN_CHUNKS))]
        opool = ctx.enter_context(tc.tile_pool(name="opool", bufs=3))
        ppool = ctx.enter_context(tc.tile_pool(name="ppool", bufs=8, space="PSUM"))

        wt = wpool.tile([K, KW * M], f16, name="wt")
        dummy = wpool.tile([K, W], f16, name="dummy")
        nc.gpsimd.memset(dummy[:], 0)
        nc.scalar.dma_start(out=wt[:], in_=wkin[:])

        xts = []
        for i, ng in enumerate(IN_CHUNKS):
            g0 = int(in_start[i])
            xt = xpools[i].tile([K, ng, WP], f16, name=f"xt{i}")
            nc.sync.dma_start(out=xt[:], in_=x[:, g0 : g0 + ng, :])
            xts.append(xt)

        # warm the PE (HAM un-throttles after ~3.4us of sustained activity)
        # while the first input chunk + weights are still in flight
        pd = ppool.tile([M, W], f32, name="pt", tag="pt")
        for _ in range(NDUMMY):
            nc.tensor.matmul(pd[:, 0:W], dummy[:, 0:M], dummy[:, 0:W],
                             start=True, stop=True)

        ot = None
        for b in range(0, NG, GB):
            groups = list(range(b, min(b + GB, NG)))
            pts = [ppool.tile([M, W], f32, name="pt", tag="pt") for _ in groups]
            for kw in range(KW):
                for g, pt in zip(groups, pts):
                    c = int(chunk_of[g])
                    gi = g - int(in_start[c])
                    nc.tensor.matmul(pt[:, 0:W], wt[:, kw * M : (kw + 1) * M],
                                     xts[c][:, gi, kw : kw + W],
                                     start=(kw == 0), stop=(kw == KW - 1))
            for g, pt in zip(groups, pts):
                oc = int(ochunk_of[g])
                o0 = int(out_start[oc])
                if ot is None:
                    ot = opool.tile([M, OUT_CHUNKS[0], W], f16, name="ot", tag="ot")
                oi = g - o0
                if g % 2 == 0:
                    nc.scalar.copy(ot[:, oi, :], pt[:])
                else:
                    nc.vector.tensor_copy(ot[:, oi, :], pt[:])
                if g == o0 + OUT_CHUNKS[oc] - 1:
                    oeng = nc.sync if oc >= len(OUT_CHUNKS) - OUT_SYNC else nc.scalar
                    oeng.dma_start(out=out[:, o0 : o0 + OUT_CHUNKS[oc], :],
                                   in_=ot[:, 0 : OUT_CHUNKS[oc], :])
                    ot = None

    nc.compile()
    return nc


def get_nc():
    if "v2" not in _CACHE:
        _CACHE["v2"] = _build_nc()
    return _CACHE["v2"]


def run(x: np.ndarray, weight: np.ndarray, **spmd_kwargs):
    """Run the conv on 8 cores; returns (out [8,16,512,512] f32, results)."""
    from concourse.bass_utils import run_bass_kernel_spmd

    x = np.asarray(x, dtype=np.float32)
    wk = _build_weights(weight)
    xps = [_build_x(x[b]) for b in range(B)]
    nc = get_nc()
    in_maps = [{"x": xps[b], "wk": wk} for b in range(B)]
    res = run_bass_kernel_spmd(nc, in_maps, list(range(B)), **spmd_kwargs)

    rows = np.array(Y0)[:, None] + np.arange(R)[None, :]   # (86, 6)
    out = np.empty((B, C_OUT, H, W), np.float32)
    for b in range(B):
        o = res.results[b]["out"].astype(np.float32)       # [96, 86, 512]
        o = o.reshape(C_OUT, R, NG, W).transpose(0, 2, 1, 3)  # [16, 86, 6, 512]
        out[b][:, rows.reshape(-1), :] = o.reshape(C_OUT, NG * R, W)
    return out, res


def kernel(x: np.ndarray, weight: np.ndarray) -> np.ndarray:
    return run(x, weight)[0]
